# revision 14
# baseline (speedup 1.0000x reference)
"""Trainium2 Bass kernel for nn_BestChangeLayer (GoL pattern search).

Math (same collapse as v1, restructured for the TRN2 timeline cost model):
for each batch b the 7x7 window W of x at (ry,rx) gets its center 3x3
replaced by each of 512 patterns p; one GoL step runs and the inner 5x5 is
compared with the target window tw.  new = [s==3] + center*[s==2] with
s = Sf(b,cell) + Sp(p,cell), so

  errors(b,p) - sum(tw) = sum_cells w*new      (w = 1-2*tw)

decomposes over "feature rows" r = (cell, v, sign, kind):

  E(b,p) = sum_r [D_r(b) == 0] * T_r(p)

where D_r is LINEAR in the window values (W cells, t cells, const):
  ringA:  D = Sf - v + 4*(1 - s*w)                 T = s*[Sp == 3-v]
  ringB:  D = Sf - v + 4*(1 - s*w) + 16*(1 - c)    T = s*[Sp == 2-v]
  inner:  D = Sf - v + 4*(1 - s*w)                 T = s*([Sp==3-v] + pc*[Sp==2-v])
(The 4(1-s*w) / 16(1-c) offsets gate w==s and c==1 into the zero test, so
no per-batch multiplier is needed: G = [D==0] is a pure one-hot.)

Pipeline per core (B=128 rows):
  - x rows ry-2..ry+4 and target rows ry-1..ry+4 land in ONE SBUF tile
    ("big") laid out so a single strided view covers both windows; ONE PE
    transpose produces stageT (84,B) + memset ones rows; D_k = C_k^T@stageT
    (3 PE matmuls, C_k constant); G_k = is_equal(D_k, 0) (3 DVE ops);
    E_ps = sum G_k^T @ T_k (3 PE matmuls into one PSUM bank).
  - negseed = -0.5*noise - E; Max + MaxIndex -> argmin index; bits via
    shift/and; bits patched into a copy of the x window rows (delta).
  - Output: x -> out DRAM->DRAM copies for the non-patch columns, a zeroed
    64-col strip for the patch region, and a SWDGE dma_scatter_add
    (prepare_only early + trigger_dma late) that adds delta into the strip.
    The prep/trigger split keeps the HWDGE+DGE latency (~1.3us) off the
    final dependent store.

Sharding: pure data parallel, batch 1024 = 8 cores x 128 rows.
"""

import os
import sys

import numpy as np

for _p in ("/opt/trn_rl_repo", "/root/.axon_site/_ro/trn_rl_repo"):
    if os.path.isdir(_p) and _p not in sys.path:
        sys.path.insert(0, _p)

import concourse.bass as bass  # noqa: E402,F401
import concourse.mybir as mybir  # noqa: E402
import concourse.tile as tile  # noqa: E402
from concourse import bacc  # noqa: E402
from concourse.bass_utils import run_bass_kernel_spmd  # noqa: E402
from concourse.masks import make_identity  # noqa: E402

N_CORES = 8
B_TOTAL = 1024
B = B_TOTAL // N_CORES  # 128 batch rows per core
H = W = 25
NPAT = 512
OW = 640  # padded out row (2560B = 10*256B, scatter stride constraint)

F32 = mybir.dt.float32
U32 = mybir.dt.uint32
I16 = mybir.dt.int16
# Low-precision dtype for matmul operands (tables, stageT, G).  All values
# involved are small integers, exact in fp8e4 (see _build_tables).
LP = mybir.dt.float8e4
LP_NP = mybir.dt.np(LP)

# ---------------------------------------------------------------------------
# Host-side constant tables
# ---------------------------------------------------------------------------


def _features():
    """Ordered feature list: (i, j, v, s, kind). kind in {'rA','rB','in'}."""
    ring, inner = [], []
    for i in range(5):
        for j in range(5):
            r, c = i + 1, j + 1
            (inner if (2 <= r <= 4 and 2 <= c <= 4) else ring).append((i, j))
    feats = []
    for (i, j) in ring:
        for v in range(4):
            for s in (1, -1):
                feats.append((i, j, v, s, "rA"))
    for (i, j) in ring:
        for v in range(3):
            for s in (1, -1):
                feats.append((i, j, v, s, "rB"))
    for (i, j) in inner:
        vmax = 1 if (i, j) == (2, 2) else 4  # center cell: Sf == 0 always
        for v in range(vmax):
            for s in (1, -1):
                feats.append((i, j, v, s, "in"))
    assert len(feats) == 290, len(feats)
    return feats


FEATS = _features()
TILE_SIZES = (128, 128, 34)
CROWS = 76  # stage rows: W window 0:49, t window 49:74, ones 74:76
ROW_ONE_A, ROW_ONE_B = 74, 75


def _row_w(u, v):
    return u * 7 + v


def _row_t(i, j):
    return 49 + 5 * i + j


def _build_tables():
    ints = np.arange(NPAT)
    shifts = np.arange(8, -1, -1)
    pats = ((ints[:, None] >> shifts[None, :]) & 1).astype(np.float32).reshape(NPAT, 3, 3)

    def cell_geom(i, j):
        r, c = i + 1, j + 1
        fixed, pat = [], []
        for dr in (-1, 0, 1):
            for dc in (-1, 0, 1):
                if dr == 0 and dc == 0:
                    continue
                u, v = r + dr, c + dc
                (pat if (2 <= u <= 4 and 2 <= v <= 4) else fixed).append((u, v))
        sp = np.zeros(NPAT, np.float32)
        for (u, v) in pat:
            sp += pats[:, u - 2, v - 2]
        pc = pats[:, r - 2, c - 2] if (2 <= r <= 4 and 2 <= c <= 4) else None
        return fixed, sp, pc, (r, c)

    geom = {(i, j): cell_geom(i, j) for i in range(5) for j in range(5)}

    Cs = [np.zeros((CROWS, 128), np.float32) for _ in range(3)]
    Ts = [np.zeros((n, NPAT), np.float32) for n in TILE_SIZES]
    f = 0
    for k, n in enumerate(TILE_SIZES):
        for col in range(n):
            i, j, v, s, kind = FEATS[f]
            f += 1
            fixed, sp, pc, (r, c) = geom[(i, j)]
            C = Cs[k]
            for (u, v2) in fixed:
                C[_row_w(u, v2), col] += 1.0
            C[_row_t(i, j), col] += 8.0 * s
            const = -v + 4 - 4 * s
            if kind == "rB":
                C[_row_w(r, c), col] += -16.0
                const += 16
            ca = const if const <= 16 else 16
            C[ROW_ONE_A, col] += ca
            C[ROW_ONE_B, col] += const - ca
            if kind == "rA":
                Ts[k][col] = s * (sp == 3 - v)
            elif kind == "rB":
                Ts[k][col] = s * (sp == 2 - v)
            else:
                Ts[k][col] = s * ((sp == 3 - v) + pc * (sp == 2 - v))
    assert f == 290

    # cgeoT3 (96, 1024): C1|C2|C3 at cols 0:384, T3 (rows 0:34) at cols 512:1024
    cgeoT3 = np.zeros((CROWS, 1024), np.float32)
    for k in range(3):
        cgeoT3[:, 128 * k:128 * (k + 1)] = Cs[k]
    cgeoT3[: TILE_SIZES[2], 512:1024] = Ts[2]
    # T12 (128, 1024): T1 | T2
    T12 = np.zeros((128, 1024), np.float32)
    T12[:, :NPAT] = Ts[0]
    T12[:, NPAT:] = Ts[1]

    # fp8 exactness check of every constant
    for arr in (cgeoT3, T12):
        assert np.array_equal(arr.astype(LP_NP).astype(np.float32), arr), (
            "constant not exact in low-precision dtype"
        )

    # Scatter-add index table rides the last 16 bytes of the T12 tensor:
    # int16 value (p%16) + 16*j at partition p, col j -- token i maps to out
    # row i for every 16-partition channel group (the SWDGE ucode reads a
    # queue-dependent group, so all groups must hold the same values).
    t12b = np.zeros((128, 1040), np.uint8)
    t12b[:, 0:1024] = T12.astype(LP_NP).view(np.uint8)
    p = np.arange(128) % 16
    j = np.arange(8)
    t12b[:, 1024::2] = (p[:, None] + 16 * j[None, :]).astype(np.uint8)
    return cgeoT3.astype(LP_NP), t12b


# ---------------------------------------------------------------------------
# Kernel builder
# ---------------------------------------------------------------------------
_CACHE = {}


def _build(ry, rx):
    # No toroidal wrap in the loaded bands / windows (true for ry=7, rx=11).
    assert 2 <= ry <= H - 5 and 2 <= rx <= W - 5, (ry, rx)
    cgeoT3_np, T12_np = _build_tables()

    nc = bacc.Bacc(None, target_bir_lowering=False)
    x_h = nc.dram_tensor("x", [B, H * W], F32, kind="ExternalInput")
    t_h = nc.dram_tensor("target", [B, H * W], F32, kind="ExternalInput")
    n_h = nc.dram_tensor("noise", [B, NPAT], F32, kind="ExternalInput")
    o_h = nc.dram_tensor("out", [B, OW], F32, kind="ExternalOutput")
    cgeoT3_h = nc.inline_tensor(cgeoT3_np, "cgeot3")
    t12_h = nc.inline_tensor(T12_np, "t12tab")

    OP = mybir.AluOpType
    po = ry * W + rx  # patch offset in the flat 625-col row

    with tile.TileContext(nc) as tc:
        with (
            tc.tile_pool(name="sb", bufs=1) as sb,
            tc.tile_pool(name="ps", bufs=1, space="PSUM") as ps,
        ):
            # ---- tiny constants / scratch (DVE, off critical path) ----
            shvec = sb.tile([B, 9], U32)
            zeros = sb.tile([B, 64], F32)
            nc.vector.memset(zeros[:], 0.0)

            # ---- input DMAs ----
            # Pool/SWDGE queue first: target rows ry-1..ry+4 (6 rows; the 6th
            # only pads the descriptor to 600B to dodge the <512B 2x penalty).
            big = sb.tile([B, 326], F32)
            nc.gpsimd.dma_start(
                out=big[:, 175:325], in_=t_h[:, (ry - 1) * W:(ry + 5) * W])
            nc.gpsimd.iota(shvec[:], pattern=[[-1, 9]], base=8, channel_multiplier=0)
            # SP: x rows ry-2..ry+4 -> big[:, 0:175]
            nc.sync.dma_start(out=big[:, 0:175], in_=x_h[:, (ry - 2) * W:(ry + 5) * W])
            # Act: geometry matrices + T3
            cgeo = sb.tile([CROWS, 1024], LP)
            nc.scalar.dma_start(out=cgeo[:], in_=cgeoT3_h[:, :])
            # SP: pattern tables T1|T2 (+ scatter idx bytes at cols
            # 1024:1040). uint8 so the 0x7f idx byte is not read as fp8 NaN;
            # table slices are bitcast back to the matmul dtype.
            t12 = sb.tile([128, 1040], mybir.dt.uint8)
            nc.sync.dma_start(out=t12[:], in_=t12_h[:, :])
            # Act: noise
            noise = sb.tile([B, NPAT], F32)
            nc.scalar.dma_start(out=noise[:], in_=n_h[:, :])

            # ---- passthrough stores (no compute deps) ----
            # DRAM->DRAM copies around the 64-col patch strip, zero strip.
            nc.sync.dma_start(out=o_h[:, 0:po], in_=x_h[:, 0:po])
            nc.scalar.dma_start(out=o_h[:, po + 64:H * W], in_=x_h[:, po + 64:H * W])
            nc.scalar.dma_start(out=o_h[:, po:po + 64], in_=zeros[:])

            # ---- identity + PE warmup (clock ramp) ----
            # Warm on the early-memset zeros tile: identity (Pool-built) is
            # only ready ~2.6us in, far too late to start the PE p-state ramp.
            ident = sb.tile([128, 128], mybir.dt.bfloat16)
            make_identity(nc, ident[:])
            warm_ps = ps.tile([64, 64], F32)
            for _ in range(2):
                nc.tensor.matmul(warm_ps[:], zeros[:, 0:64], zeros[:, 0:64],
                                 start=True, stop=True)

            # ---- delta strip: x window cols with the patch bits ----
            # big cols 61:125 = x rows ry..ry+2 from col rx (64 consecutive)
            dbase = 2 * W + rx
            delta = sb.tile([B, 75], F32)
            nc.vector.tensor_copy(out=delta[:, 0:64], in_=big[:, dbase:dbase + 64])

            # ---- scatter-add prep (descriptors only; fires via trigger) ----
            # sem must be the Tile DMASW lane this prep is assigned (it is the
            # 2nd Pool-queue DMA inst emitted -> DMASW1): the drain at context
            # exit waits on that lane, and walrus encodes exactly one DMA sem.
            import bass_rust as _br
            dma_sem = tc.sems[_br.PROC_NAMES.index("DMASW1")]
            nc.gpsimd.dma_scatter_add(
                o_h[:, po:po + 64],
                delta[:, 0:64].rearrange("b (x c) -> b x c", x=1),
                t12[:, 1024:1040].bitcast(I16),
                128,
                128,
                64,
                elem_step=OW,
                prepare_only=True,
                sem=dma_sem,
            )

            # ---- staging: extract both windows into a contiguous bf16 tile
            # (matmul stationary APs allow only one free dim, so the strided
            # window view cannot feed the transpose directly), then one
            # transpose + PSUM->SBUF convert to the matmul dtype.
            stage = sb.tile([B, CROWS], mybir.dt.bfloat16)
            nc.vector.memset(stage[:, 74:76], 1.0)  # ones rows
            bigv = big[:, 0:325].rearrange("b (r c) -> b r c", r=13)
            ws = stage[:, 0:49].rearrange("b (r c) -> b r c", r=7)
            nc.vector.tensor_copy(out=ws[:], in_=bigv[:, 0:7, rx - 2:rx + 5])
            ts = stage[:, 49:74].rearrange("b (r c) -> b r c", r=5)
            nc.vector.tensor_copy(out=ts[:], in_=bigv[:, 7:12, rx - 1:rx + 4])
            ps_stage = ps.tile([CROWS, B], mybir.dt.bfloat16)
            nc.tensor.transpose(out=ps_stage[:], in_=stage[:], identity=ident[:])
            stageT = sb.tile([CROWS, B], LP)
            nc.vector.tensor_copy(out=stageT[:], in_=ps_stage[:])

            # ---- D matmuls + one-hot G + E matmuls ----
            D12 = ps.tile([128, 2 * B], F32)
            D3 = ps.tile([TILE_SIZES[2], B], F32)
            E_ps = ps.tile([B, NPAT], F32)
            # D1/D2 share one PSUM tile as column blocks so a single is_equal
            # produces G1|G2 — fewer DVE ops, and the all-G-before-all-E
            # linear order keeps tile's sem-wait coarsening from serializing
            # G ops behind E matmuls.
            n3 = TILE_SIZES[2]
            nc.tensor.matmul(D12[:, 0:B], cgeo[:, 0:128], stageT[:],
                             start=True, stop=True)
            nc.tensor.matmul(D12[:, B:2 * B], cgeo[:, 128:256], stageT[:],
                             start=True, stop=True)
            nc.tensor.matmul(D3[:], cgeo[:, 256:256 + n3], stageT[:],
                             start=True, stop=True)
            G12 = sb.tile([128, 2 * B], LP)
            nc.vector.tensor_scalar(G12[:], D12[:], 0.0, None, OP.is_equal)
            G3 = sb.tile([n3, B], LP)
            nc.vector.tensor_scalar(G3[:], D3[:], 0.0, None, OP.is_equal)
            stats = ((G12[:, 0:B], t12[:, 0:NPAT].bitcast(LP)),
                     (G12[:, B:2 * B], t12[:, NPAT:2 * NPAT].bitcast(LP)),
                     (G3[:], cgeo[0:n3, 512:1024]))
            for k, (g, tab) in enumerate(stats):
                nc.tensor.matmul(E_ps[:], g, tab,
                                 start=(k == 0), stop=(k == 2))


            # ---- argmin via negate/max/max_index (reference tie semantics) ----
            negseed = sb.tile([B, NPAT], F32)
            nc.vector.scalar_tensor_tensor(
                out=negseed[:], in0=noise[:], scalar=-0.5, in1=E_ps[:],
                op0=OP.mult, op1=OP.subtract,
            )
            mx8 = sb.tile([B, 8], F32)
            nc.vector.max(out=mx8[:], in_=negseed[:])
            idx8 = sb.tile([B, 8], U32)
            nc.vector.max_index(out=idx8[:], in_max=mx8[:], in_values=negseed[:])

            # ---- bits of the argmin into the delta strip ----
            shd = sb.tile([B, 9], U32)
            nc.vector.tensor_tensor(
                out=shd[:], in0=idx8[:, 0:1].to_broadcast([B, 9]), in1=shvec[:],
                op=OP.logical_shift_right,
            )
            bitu = sb.tile([B, 9], U32)
            nc.vector.tensor_scalar(bitu[:], shd[:], 1, None, OP.bitwise_and)
            dview = delta[:].rearrange("b (r c) -> b r c", r=3)
            nc.vector.tensor_copy(
                out=dview[:, :, 0:3], in_=bitu[:].rearrange("b (r c) -> b r c", r=3))

            # ---- fire the patch scatter (the context-exit drain waits on
            # the DMASW1 lane, so the kernel cannot end before it lands) ----
            nc.gpsimd.trigger_dma(count=None)

    nc.finalize()
    return nc


def _get(ry, rx):
    key = (ry, rx)
    if key not in _CACHE:
        _CACHE[key] = _build(ry, rx)
    return _CACHE[key]


def kernel_with_results(x, target, noise, ry, rx, trace=False):
    x = np.ascontiguousarray(np.asarray(x, dtype=np.float32))
    target = np.ascontiguousarray(np.asarray(target, dtype=np.float32))
    noise = np.ascontiguousarray(np.asarray(noise, dtype=np.float32))
    ry, rx = int(ry), int(rx)
    Btot = x.shape[0]
    assert Btot == B_TOTAL and x.shape == (Btot, 1, H, W), x.shape

    nc = _get(ry, rx)
    xs = x.reshape(Btot, H * W)
    ts = target.reshape(Btot, H * W)
    in_maps = [
        {
            "x": xs[c * B:(c + 1) * B],
            "target": ts[c * B:(c + 1) * B],
            "noise": noise[c * B:(c + 1) * B],
        }
        for c in range(N_CORES)
    ]
    res = run_bass_kernel_spmd(nc, in_maps, core_ids=list(range(N_CORES)), trace=trace)
    out = np.concatenate(
        [res.results[c]["out"][:, :H * W] for c in range(N_CORES)], axis=0)
    return out.reshape(Btot, 1, H, W).astype(np.float32), res


def kernel(x, target, noise, ry, rx):
    out, _ = kernel_with_results(x, target, noise, ry, rx)
    return out


# revision 19
# speedup vs baseline: 1.0161x; 1.0161x over previous
"""Trainium2 Bass kernel for nn_BestChangeLayer (GoL pattern search).

Math (same collapse as v1, restructured for the TRN2 timeline cost model):
for each batch b the 7x7 window W of x at (ry,rx) gets its center 3x3
replaced by each of 512 patterns p; one GoL step runs and the inner 5x5 is
compared with the target window tw.  new = [s==3] + center*[s==2] with
s = Sf(b,cell) + Sp(p,cell), so

  errors(b,p) - sum(tw) = sum_cells w*new      (w = 1-2*tw)

decomposes over "feature rows" r = (cell, v, sign, kind):

  E(b,p) = sum_r [D_r(b) == 0] * T_r(p)

where D_r is LINEAR in the window values (W cells, t cells, const):
  ringA:  D = Sf - v + 4*(1 - s*w)                 T = s*[Sp == 3-v]
  ringB:  D = Sf - v + 4*(1 - s*w) + 16*(1 - c)    T = s*[Sp == 2-v]
  inner:  D = Sf - v + 4*(1 - s*w)                 T = s*([Sp==3-v] + pc*[Sp==2-v])
(The 4(1-s*w) / 16(1-c) offsets gate w==s and c==1 into the zero test, so
no per-batch multiplier is needed: G = [D==0] is a pure one-hot.)

Pipeline per core (B=128 rows):
  - x rows ry-2..ry+4 and target rows ry-1..ry+4 land in ONE SBUF tile
    ("big"); two DVE copies extract the 7x7/5x5 windows into a contiguous
    bf16 stage (B,76) with inline ones columns; ONE PE transpose produces
    stageT; D = C^T@stageT (3 PE matmuls into packed PSUM, C constant fp8);
    G = is_equal(D, 0) (2 DVE ops, fp8 out);
    E_ps = sum G_k^T @ T_k (3 fp8 PE matmuls into one PSUM bank).
  - negseed = -0.5*noise - E; Max + MaxIndex -> argmin index; bits via
    shift/and; bits patched into a copy of the x window rows (delta).
  - Output: x -> out DRAM->DRAM copies for the non-patch columns, a zeroed
    64-col strip for the patch region, and a SWDGE dma_scatter_add
    (prepare_only early + trigger_dma late) that adds delta into the strip.
    The prep/trigger split keeps the HWDGE+DGE latency (~1.3us) off the
    final dependent store.

Sharding: pure data parallel, batch 1024 = 8 cores x 128 rows.
"""

import os
import sys

import numpy as np

for _p in ("/opt/trn_rl_repo", "/root/.axon_site/_ro/trn_rl_repo"):
    if os.path.isdir(_p) and _p not in sys.path:
        sys.path.insert(0, _p)

import concourse.bass as bass  # noqa: E402,F401
import concourse.mybir as mybir  # noqa: E402
import concourse.tile as tile  # noqa: E402
from concourse import bacc  # noqa: E402
from concourse.bass_utils import run_bass_kernel_spmd  # noqa: E402
from concourse.masks import make_identity  # noqa: E402

N_CORES = 8
B_TOTAL = 1024
B = B_TOTAL // N_CORES  # 128 batch rows per core
H = W = 25
NPAT = 512
OW = 640  # padded out row (2560B = 10*256B, scatter stride constraint)

F32 = mybir.dt.float32
U32 = mybir.dt.uint32
I16 = mybir.dt.int16
# Low-precision dtype for matmul operands (tables, stageT, G).  All values
# involved are small integers, exact in fp8e4 (see _build_tables).
LP = mybir.dt.float8e4
LP_NP = mybir.dt.np(LP)

# ---------------------------------------------------------------------------
# Host-side constant tables
# ---------------------------------------------------------------------------


def _features():
    """Ordered feature list: (i, j, v, s, kind). kind in {'rA','rB','in'}."""
    ring, inner = [], []
    for i in range(5):
        for j in range(5):
            r, c = i + 1, j + 1
            (inner if (2 <= r <= 4 and 2 <= c <= 4) else ring).append((i, j))
    feats = []
    for (i, j) in ring:
        for v in range(4):
            for s in (1, -1):
                feats.append((i, j, v, s, "rA"))
    for (i, j) in ring:
        for v in range(3):
            for s in (1, -1):
                feats.append((i, j, v, s, "rB"))
    for (i, j) in inner:
        vmax = 1 if (i, j) == (2, 2) else 4  # center cell: Sf == 0 always
        for v in range(vmax):
            for s in (1, -1):
                feats.append((i, j, v, s, "in"))
    assert len(feats) == 290, len(feats)
    return feats


FEATS = _features()
TILE_SIZES = (128, 128, 34)
CROWS = 76  # stage rows: W window 0:49, t window 49:74, ones 74:76
ROW_ONE_A, ROW_ONE_B = 74, 75


def _row_w(u, v):
    return u * 7 + v


def _row_t(i, j):
    return 49 + 5 * i + j


def _build_tables():
    ints = np.arange(NPAT)
    shifts = np.arange(8, -1, -1)
    pats = ((ints[:, None] >> shifts[None, :]) & 1).astype(np.float32).reshape(NPAT, 3, 3)

    def cell_geom(i, j):
        r, c = i + 1, j + 1
        fixed, pat = [], []
        for dr in (-1, 0, 1):
            for dc in (-1, 0, 1):
                if dr == 0 and dc == 0:
                    continue
                u, v = r + dr, c + dc
                (pat if (2 <= u <= 4 and 2 <= v <= 4) else fixed).append((u, v))
        sp = np.zeros(NPAT, np.float32)
        for (u, v) in pat:
            sp += pats[:, u - 2, v - 2]
        pc = pats[:, r - 2, c - 2] if (2 <= r <= 4 and 2 <= c <= 4) else None
        return fixed, sp, pc, (r, c)

    geom = {(i, j): cell_geom(i, j) for i in range(5) for j in range(5)}

    Cs = [np.zeros((CROWS, 128), np.float32) for _ in range(3)]
    Ts = [np.zeros((n, NPAT), np.float32) for n in TILE_SIZES]
    f = 0
    for k, n in enumerate(TILE_SIZES):
        for col in range(n):
            i, j, v, s, kind = FEATS[f]
            f += 1
            fixed, sp, pc, (r, c) = geom[(i, j)]
            C = Cs[k]
            for (u, v2) in fixed:
                C[_row_w(u, v2), col] += 1.0
            C[_row_t(i, j), col] += 8.0 * s
            const = -v + 4 - 4 * s
            if kind == "rB":
                C[_row_w(r, c), col] += -16.0
                const += 16
            ca = const if const <= 16 else 16
            C[ROW_ONE_A, col] += ca
            C[ROW_ONE_B, col] += const - ca
            if kind == "rA":
                Ts[k][col] = s * (sp == 3 - v)
            elif kind == "rB":
                Ts[k][col] = s * (sp == 2 - v)
            else:
                Ts[k][col] = s * ((sp == 3 - v) + pc * (sp == 2 - v))
    assert f == 290

    # cgeoT3 (96, 1024): C1|C2|C3 at cols 0:384, T3 (rows 0:34) at cols 512:1024
    cgeoT3 = np.zeros((CROWS, 1024), np.float32)
    for k in range(3):
        cgeoT3[:, 128 * k:128 * (k + 1)] = Cs[k]
    cgeoT3[: TILE_SIZES[2], 512:1024] = Ts[2]
    # T12 (128, 1024): T1 | T2
    T12 = np.zeros((128, 1024), np.float32)
    T12[:, :NPAT] = Ts[0]
    T12[:, NPAT:] = Ts[1]

    # fp8 exactness check of every constant
    for arr in (cgeoT3, T12):
        assert np.array_equal(arr.astype(LP_NP).astype(np.float32), arr), (
            "constant not exact in low-precision dtype"
        )

    # Scatter-add index table rides the last 16 bytes of the T12 tensor:
    # int16 value (p%16) + 16*j at partition p, col j -- token i maps to out
    # row i for every 16-partition channel group (the SWDGE ucode reads a
    # queue-dependent group, so all groups must hold the same values).
    t12b = np.zeros((128, 1040), np.uint8)
    t12b[:, 0:1024] = T12.astype(LP_NP).view(np.uint8)
    p = np.arange(128) % 16
    j = np.arange(8)
    t12b[:, 1024::2] = (p[:, None] + 16 * j[None, :]).astype(np.uint8)
    return cgeoT3.astype(LP_NP), t12b


# ---------------------------------------------------------------------------
# Kernel builder
# ---------------------------------------------------------------------------
_CACHE = {}


def _build(ry, rx):
    # No toroidal wrap in the loaded bands / windows (true for ry=7, rx=11).
    assert 2 <= ry <= H - 5 and 2 <= rx <= W - 5, (ry, rx)
    cgeoT3_np, T12_np = _build_tables()

    nc = bacc.Bacc(None, target_bir_lowering=False)
    x_h = nc.dram_tensor("x", [B, H * W], F32, kind="ExternalInput")
    t_h = nc.dram_tensor("target", [B, H * W], F32, kind="ExternalInput")
    n_h = nc.dram_tensor("noise", [B, NPAT], F32, kind="ExternalInput")
    o_h = nc.dram_tensor("out", [B, OW], F32, kind="ExternalOutput")
    cgeoT3_h = nc.inline_tensor(cgeoT3_np, "cgeot3")
    t12_h = nc.inline_tensor(T12_np, "t12tab")

    OP = mybir.AluOpType
    po = ry * W + rx  # patch offset in the flat 625-col row

    with tile.TileContext(nc) as tc:
        with (
            tc.tile_pool(name="sb", bufs=1) as sb,
            tc.tile_pool(name="ps", bufs=1, space="PSUM") as ps,
        ):
            # ---- tiny constants / scratch (DVE, off critical path) ----
            shvec = sb.tile([B, 9], U32)
            zeros = sb.tile([B, 64], F32)
            nc.vector.memset(zeros[:], 0.0)

            # ---- input DMAs ----
            # Pool/SWDGE queue first: target rows ry-1..ry+4 (6 rows; the 6th
            # only pads the descriptor to 600B to dodge the <512B 2x penalty).
            big = sb.tile([B, 326], F32)
            nc.gpsimd.dma_start(
                out=big[:, 175:325], in_=t_h[:, (ry - 1) * W:(ry + 5) * W])
            nc.gpsimd.iota(shvec[:], pattern=[[-1, 9]], base=8, channel_multiplier=0)
            # SP: x rows ry-2..ry+4 -> big[:, 0:175]
            nc.sync.dma_start(out=big[:, 0:175], in_=x_h[:, (ry - 2) * W:(ry + 5) * W])
            # Act: geometry matrices + T3
            cgeo = sb.tile([CROWS, 1024], LP)
            nc.scalar.dma_start(out=cgeo[:], in_=cgeoT3_h[:, :])
            # SP: pattern tables T1|T2 (+ scatter idx bytes at cols
            # 1024:1040). uint8 so the 0x7f idx byte is not read as fp8 NaN;
            # table slices are bitcast back to the matmul dtype.
            t12 = sb.tile([128, 1040], mybir.dt.uint8)
            nc.sync.dma_start(out=t12[:], in_=t12_h[:, :])
            # Act: noise
            noise = sb.tile([B, NPAT], F32)
            nc.scalar.dma_start(out=noise[:], in_=n_h[:, :])

            # ---- passthrough stores (no compute deps) ----
            # DRAM->DRAM copies around the 64-col patch strip, zero strip.
            nc.sync.dma_start(out=o_h[:, 0:po], in_=x_h[:, 0:po])
            nc.scalar.dma_start(out=o_h[:, po + 64:H * W], in_=x_h[:, po + 64:H * W])
            nc.scalar.dma_start(out=o_h[:, po:po + 64], in_=zeros[:])

            # ---- identity + PE warmup (clock ramp) ----
            # Warm on the early-memset zeros tile: identity (Pool-built) is
            # only ready ~2.6us in, far too late to start the PE p-state ramp.
            ident = sb.tile([128, 128], mybir.dt.bfloat16)
            make_identity(nc, ident[:])
            warm_ps = ps.tile([64, 64], F32)
            for _ in range(2):
                nc.tensor.matmul(warm_ps[:], zeros[:, 0:64], zeros[:, 0:64],
                                 start=True, stop=True)

            # ---- delta strip: x window cols with the patch bits ----
            # big cols 61:125 = x rows ry..ry+2 from col rx (64 consecutive)
            dbase = 2 * W + rx
            delta = sb.tile([B, 75], F32)
            nc.vector.tensor_copy(out=delta[:, 0:64], in_=big[:, dbase:dbase + 64])

            # ---- scatter-add prep (descriptors only; fires via trigger) ----
            # sem must be the Tile DMASW lane this prep is assigned (it is the
            # 2nd Pool-queue DMA inst emitted -> DMASW1): the drain at context
            # exit waits on that lane, and walrus encodes exactly one DMA sem.
            import bass_rust as _br
            dma_sem = tc.sems[_br.PROC_NAMES.index("DMASW1")]
            nc.gpsimd.dma_scatter_add(
                o_h[:, po:po + 64],
                delta[:, 0:64].rearrange("b (x c) -> b x c", x=1),
                t12[:, 1024:1040].bitcast(I16),
                128,
                128,
                64,
                elem_step=OW,
                prepare_only=True,
                sem=dma_sem,
            )

            # ---- staging: extract both windows into a contiguous bf16 tile
            # (matmul stationary APs allow only one free dim, so the strided
            # window view cannot feed the transpose directly), then one
            # transpose + PSUM->SBUF convert to the matmul dtype.
            stage = sb.tile([B, CROWS], mybir.dt.bfloat16)
            nc.vector.memset(stage[:, 74:76], 1.0)  # ones rows
            bigv = big[:, 0:325].rearrange("b (r c) -> b r c", r=13)
            ws = stage[:, 0:49].rearrange("b (r c) -> b r c", r=7)
            nc.vector.tensor_copy(out=ws[:], in_=bigv[:, 0:7, rx - 2:rx + 5])
            ts = stage[:, 49:74].rearrange("b (r c) -> b r c", r=5)
            nc.vector.tensor_copy(out=ts[:], in_=bigv[:, 7:12, rx - 1:rx + 4])
            ps_stage = ps.tile([CROWS, B], mybir.dt.bfloat16)
            nc.tensor.transpose(out=ps_stage[:], in_=stage[:], identity=ident[:])
            stageT = sb.tile([CROWS, B], LP)
            nc.vector.tensor_copy(out=stageT[:], in_=ps_stage[:])

            # ---- D matmuls + one-hot G + E matmuls ----
            D12 = ps.tile([128, 2 * B], F32)
            D3 = ps.tile([TILE_SIZES[2], B], F32)
            E_ps = ps.tile([B, NPAT], F32)
            # D1/D2 share one PSUM tile as column blocks so a single is_equal
            # produces G1|G2 — fewer DVE ops, and the all-G-before-all-E
            # linear order keeps tile's sem-wait coarsening from serializing
            # G ops behind E matmuls.
            n3 = TILE_SIZES[2]
            nc.tensor.matmul(D12[:, 0:B], cgeo[:, 0:128], stageT[:],
                             start=True, stop=True)
            nc.tensor.matmul(D12[:, B:2 * B], cgeo[:, 128:256], stageT[:],
                             start=True, stop=True)
            nc.tensor.matmul(D3[:], cgeo[:, 256:256 + n3], stageT[:],
                             start=True, stop=True)
            G12 = sb.tile([128, 2 * B], LP)
            nc.vector.tensor_scalar(G12[:], D12[:], 0.0, None, OP.is_equal)
            G3 = sb.tile([n3, B], LP)
            nc.vector.tensor_scalar(G3[:], D3[:], 0.0, None, OP.is_equal)
            # DoubleRow fp8: one matmul computes G1^T@T1 + G2^T@T2 (the
            # stationary/moving free dims each hold the two operand halves)
            # at 0.5 cycles/row.
            nc.tensor.matmul(
                E_ps[:],
                G12[:].rearrange("f (two b) -> f two b", two=2),
                t12[:, 0:2 * NPAT].bitcast(LP).rearrange(
                    "f (two n) -> f two n", two=2),
                perf_mode=mybir.MatmulPerfMode.DoubleRow,
                start=True, stop=False)
            nc.tensor.matmul(E_ps[:], G3[:], cgeo[0:n3, 512:1024],
                             start=False, stop=True)


            # ---- argmin via negate/max/max_index (reference tie semantics) ----
            negseed = sb.tile([B, NPAT], F32)
            nc.vector.scalar_tensor_tensor(
                out=negseed[:], in0=noise[:], scalar=-0.5, in1=E_ps[:],
                op0=OP.mult, op1=OP.subtract,
            )
            mx8 = sb.tile([B, 8], F32)
            nc.vector.max(out=mx8[:], in_=negseed[:])
            idx8 = sb.tile([B, 8], U32)
            nc.vector.max_index(out=idx8[:], in_max=mx8[:], in_values=negseed[:])

            # ---- bits of the argmin into the delta strip ----
            shd = sb.tile([B, 9], U32)
            nc.vector.tensor_tensor(
                out=shd[:], in0=idx8[:, 0:1].to_broadcast([B, 9]), in1=shvec[:],
                op=OP.logical_shift_right,
            )
            bitu = sb.tile([B, 9], U32)
            nc.vector.tensor_scalar(bitu[:], shd[:], 1, None, OP.bitwise_and)
            dview = delta[:].rearrange("b (r c) -> b r c", r=3)
            nc.vector.tensor_copy(
                out=dview[:, :, 0:3], in_=bitu[:].rearrange("b (r c) -> b r c", r=3))

            # ---- fire the patch scatter (the context-exit drain waits on
            # the DMASW1 lane, so the kernel cannot end before it lands) ----
            nc.gpsimd.trigger_dma(count=None)

    nc.finalize()
    return nc


def _get(ry, rx):
    key = (ry, rx)
    if key not in _CACHE:
        _CACHE[key] = _build(ry, rx)
    return _CACHE[key]


def kernel_with_results(x, target, noise, ry, rx, trace=False):
    x = np.ascontiguousarray(np.asarray(x, dtype=np.float32))
    target = np.ascontiguousarray(np.asarray(target, dtype=np.float32))
    noise = np.ascontiguousarray(np.asarray(noise, dtype=np.float32))
    ry, rx = int(ry), int(rx)
    Btot = x.shape[0]
    assert Btot == B_TOTAL and x.shape == (Btot, 1, H, W), x.shape

    nc = _get(ry, rx)
    xs = x.reshape(Btot, H * W)
    ts = target.reshape(Btot, H * W)
    in_maps = [
        {
            "x": xs[c * B:(c + 1) * B],
            "target": ts[c * B:(c + 1) * B],
            "noise": noise[c * B:(c + 1) * B],
        }
        for c in range(N_CORES)
    ]
    res = run_bass_kernel_spmd(nc, in_maps, core_ids=list(range(N_CORES)), trace=trace)
    out = np.concatenate(
        [res.results[c]["out"][:, :H * W] for c in range(N_CORES)], axis=0)
    return out.reshape(Btot, 1, H, W).astype(np.float32), res


def kernel(x, target, noise, ry, rx):
    out, _ = kernel_with_results(x, target, noise, ry, rx)
    return out


# revision 27
# speedup vs baseline: 1.0201x; 1.0039x over previous
"""Trainium2 Bass kernel for nn_BestChangeLayer (GoL pattern search).

Math (same collapse as v1, restructured for the TRN2 timeline cost model):
for each batch b the 7x7 window W of x at (ry,rx) gets its center 3x3
replaced by each of 512 patterns p; one GoL step runs and the inner 5x5 is
compared with the target window tw.  new = [s==3] + center*[s==2] with
s = Sf(b,cell) + Sp(p,cell), so

  errors(b,p) - sum(tw) = sum_cells w*new      (w = 1-2*tw)

decomposes over "feature rows" r = (cell, v, sign, kind):

  E(b,p) = sum_r [D_r(b) == 0] * T_r(p)

where D_r is LINEAR in the window values (W cells, t cells, const):
  ringA:  D = Sf - v + 4*(1 - s*w)                 T = s*[Sp == 3-v]
  ringB:  D = Sf - v + 4*(1 - s*w) + 16*(1 - c)    T = s*[Sp == 2-v]
  inner:  D = Sf - v + 4*(1 - s*w)                 T = s*([Sp==3-v] + pc*[Sp==2-v])
(The 4(1-s*w) / 16(1-c) offsets gate w==s and c==1 into the zero test, so
no per-batch multiplier is needed: G = [D==0] is a pure one-hot.)

Pipeline per core (B=128 rows):
  - x rows ry-2..ry+4 and target rows ry-1..ry+4 land in ONE SBUF tile
    ("big"); two DVE copies extract the 7x7/5x5 windows into a contiguous
    bf16 stage (B,76) with inline ones columns; ONE PE transpose produces
    stageT; D = C^T@stageT (3 PE matmuls into packed PSUM, C constant fp8);
    G = is_equal(D, 0) (2 DVE ops, fp8 out);
    E_ps = sum G_k^T @ T_k (3 fp8 PE matmuls into one PSUM bank).
  - negseed = -0.5*noise - E; Max + MaxIndex -> argmin index; bits via
    shift/and; bits patched into a copy of the x window rows (delta).
  - Output: x -> out DRAM->DRAM copies for the non-patch columns, a zeroed
    64-col strip for the patch region, and a SWDGE dma_scatter_add
    (prepare_only early + trigger_dma late) that adds delta into the strip.
    The prep/trigger split keeps the HWDGE+DGE latency (~1.3us) off the
    final dependent store.

Sharding: pure data parallel, batch 1024 = 8 cores x 128 rows.
"""

import os
import sys

import numpy as np

for _p in ("/opt/trn_rl_repo", "/root/.axon_site/_ro/trn_rl_repo"):
    if os.path.isdir(_p) and _p not in sys.path:
        sys.path.insert(0, _p)

import concourse.bass as bass  # noqa: E402,F401
import concourse.mybir as mybir  # noqa: E402
import concourse.tile as tile  # noqa: E402
from concourse import bacc  # noqa: E402
from concourse.bass_utils import run_bass_kernel_spmd  # noqa: E402
from concourse.masks import make_identity  # noqa: E402

N_CORES = 8
B_TOTAL = 1024
B = B_TOTAL // N_CORES  # 128 batch rows per core
H = W = 25
NPAT = 512
OW = 640  # padded out row (2560B = 10*256B, scatter stride constraint)

F32 = mybir.dt.float32
U32 = mybir.dt.uint32
I16 = mybir.dt.int16
# Low-precision dtype for matmul operands (tables, stageT, G).  All values
# involved are small integers, exact in fp8e4 (see _build_tables).
LP = mybir.dt.float8e4
LP_NP = mybir.dt.np(LP)

# ---------------------------------------------------------------------------
# Host-side constant tables
# ---------------------------------------------------------------------------


def _features():
    """Ordered feature list: (i, j, v, s, kind). kind in {'rA','rB','in'}."""
    ring, inner = [], []
    for i in range(5):
        for j in range(5):
            r, c = i + 1, j + 1
            (inner if (2 <= r <= 4 and 2 <= c <= 4) else ring).append((i, j))
    feats = []
    for (i, j) in ring:
        for v in range(4):
            for s in (1, -1):
                feats.append((i, j, v, s, "rA"))
    for (i, j) in ring:
        for v in range(3):
            for s in (1, -1):
                feats.append((i, j, v, s, "rB"))
    for (i, j) in inner:
        vmax = 1 if (i, j) == (2, 2) else 4  # center cell: Sf == 0 always
        for v in range(vmax):
            for s in (1, -1):
                feats.append((i, j, v, s, "in"))
    assert len(feats) == 290, len(feats)
    return feats


FEATS = _features()
TILE_SIZES = (128, 128, 34)
CROWS = 76  # stage rows: W window 0:49, t window 49:74, ones 74:76
ROW_ONE_A, ROW_ONE_B = 74, 75


def _row_w(u, v):
    return u * 7 + v


def _row_t(i, j):
    return 49 + 5 * i + j


def _build_tables():
    ints = np.arange(NPAT)
    shifts = np.arange(8, -1, -1)
    pats = ((ints[:, None] >> shifts[None, :]) & 1).astype(np.float32).reshape(NPAT, 3, 3)

    def cell_geom(i, j):
        r, c = i + 1, j + 1
        fixed, pat = [], []
        for dr in (-1, 0, 1):
            for dc in (-1, 0, 1):
                if dr == 0 and dc == 0:
                    continue
                u, v = r + dr, c + dc
                (pat if (2 <= u <= 4 and 2 <= v <= 4) else fixed).append((u, v))
        sp = np.zeros(NPAT, np.float32)
        for (u, v) in pat:
            sp += pats[:, u - 2, v - 2]
        pc = pats[:, r - 2, c - 2] if (2 <= r <= 4 and 2 <= c <= 4) else None
        return fixed, sp, pc, (r, c)

    geom = {(i, j): cell_geom(i, j) for i in range(5) for j in range(5)}

    Cs = [np.zeros((CROWS, 128), np.float32) for _ in range(3)]
    Ts = [np.zeros((n, NPAT), np.float32) for n in TILE_SIZES]
    f = 0
    for k, n in enumerate(TILE_SIZES):
        for col in range(n):
            i, j, v, s, kind = FEATS[f]
            f += 1
            fixed, sp, pc, (r, c) = geom[(i, j)]
            C = Cs[k]
            for (u, v2) in fixed:
                C[_row_w(u, v2), col] += 1.0
            C[_row_t(i, j), col] += 8.0 * s
            const = -v + 4 - 4 * s
            if kind == "rB":
                C[_row_w(r, c), col] += -16.0
                const += 16
            ca = const if const <= 16 else 16
            C[ROW_ONE_A, col] += ca
            C[ROW_ONE_B, col] += const - ca
            if kind == "rA":
                Ts[k][col] = s * (sp == 3 - v)
            elif kind == "rB":
                Ts[k][col] = s * (sp == 2 - v)
            else:
                Ts[k][col] = s * ((sp == 3 - v) + pc * (sp == 2 - v))
    assert f == 290

    # cgeoT3 (76, 1536): C1|C2|C3 at cols 0:384, T3 (rows 0:34) at cols
    # 512:1024, zeros at 1024:1536 (zero half of the E3 DoubleRow pair)
    cgeoT3 = np.zeros((CROWS, 1536), np.float32)
    for k in range(3):
        cgeoT3[:, 128 * k:128 * (k + 1)] = Cs[k]
    cgeoT3[: TILE_SIZES[2], 512:1024] = Ts[2]
    # T12 (128, 1024): T1 | T2
    T12 = np.zeros((128, 1024), np.float32)
    T12[:, :NPAT] = Ts[0]
    T12[:, NPAT:] = Ts[1]

    # fp8 exactness check of every constant
    for arr in (cgeoT3, T12):
        assert np.array_equal(arr.astype(LP_NP).astype(np.float32), arr), (
            "constant not exact in low-precision dtype"
        )

    # Scatter-add index table rides the last 16 bytes of the T12 tensor:
    # int16 value (p%16) + 16*j at partition p, col j -- token i maps to out
    # row i for every 16-partition channel group (the SWDGE ucode reads a
    # queue-dependent group, so all groups must hold the same values).
    t12b = np.zeros((128, 1040), np.uint8)
    t12b[:, 0:1024] = T12.astype(LP_NP).view(np.uint8)
    p = np.arange(128) % 16
    j = np.arange(8)
    t12b[:, 1024::2] = (p[:, None] + 16 * j[None, :]).astype(np.uint8)
    return cgeoT3.astype(LP_NP), t12b


# ---------------------------------------------------------------------------
# Kernel builder
# ---------------------------------------------------------------------------
_CACHE = {}


def _build(ry, rx):
    # No toroidal wrap in the loaded bands / windows (true for ry=7, rx=11).
    assert 2 <= ry <= H - 5 and 2 <= rx <= W - 5, (ry, rx)
    cgeoT3_np, T12_np = _build_tables()

    nc = bacc.Bacc(None, target_bir_lowering=False)
    x_h = nc.dram_tensor("x", [B, H * W], F32, kind="ExternalInput")
    t_h = nc.dram_tensor("target", [B, H * W], F32, kind="ExternalInput")
    n_h = nc.dram_tensor("noise", [B, NPAT], F32, kind="ExternalInput")
    o_h = nc.dram_tensor("out", [B, OW], F32, kind="ExternalOutput")
    cgeoT3_h = nc.inline_tensor(cgeoT3_np, "cgeot3")
    t12_h = nc.inline_tensor(T12_np, "t12tab")

    OP = mybir.AluOpType
    po = ry * W + rx  # patch offset in the flat 625-col row

    with tile.TileContext(nc) as tc:
        with (
            tc.tile_pool(name="sb", bufs=1) as sb,
            tc.tile_pool(name="ps", bufs=1, space="PSUM") as ps,
        ):
            # ---- tiny constants / scratch (DVE, off critical path) ----
            shvec = sb.tile([B, 9], U32)
            zeros = sb.tile([B, 64], F32)
            nc.vector.memset(zeros[:], 0.0)
            n3 = TILE_SIZES[2]
            G3 = sb.tile([n3, 2 * B], LP)
            nc.vector.memset(G3[:, B:2 * B], 0.0)  # zero half of E3 DoubleRow

            # ---- input DMAs ----
            # Pool/SWDGE queue first: target rows ry-1..ry+4 (6 rows; the 6th
            # only pads the descriptor to 600B to dodge the <512B 2x penalty).
            big = sb.tile([B, 326], F32)
            nc.gpsimd.dma_start(
                out=big[:, 175:325], in_=t_h[:, (ry - 1) * W:(ry + 5) * W])
            nc.gpsimd.iota(shvec[:], pattern=[[-1, 9]], base=8, channel_multiplier=0)
            # SP: x rows ry-2..ry+4 -> big[:, 0:175]
            nc.sync.dma_start(out=big[:, 0:175], in_=x_h[:, (ry - 2) * W:(ry + 5) * W])
            # Act: geometry matrices + T3
            cgeo = sb.tile([CROWS, 1536], LP)
            nc.scalar.dma_start(out=cgeo[:], in_=cgeoT3_h[:, :])
            # SP: pattern tables T1|T2 (+ scatter idx bytes at cols
            # 1024:1040). uint8 so the 0x7f idx byte is not read as fp8 NaN;
            # table slices are bitcast back to the matmul dtype.
            t12 = sb.tile([128, 1040], mybir.dt.uint8)
            noise = sb.tile([B, NPAT], F32)
            # SP: noise first (it co-gates negseed with the E-matmul end)
            nc.sync.dma_start(out=noise[:], in_=n_h[:, :])
            # Act: tables
            nc.scalar.dma_start(out=t12[:], in_=t12_h[:, :])

            # ---- passthrough stores (no compute deps) ----
            # DRAM->DRAM copies around the 64-col patch strip, zero strip.
            nc.sync.dma_start(out=o_h[:, 0:po], in_=x_h[:, 0:po])
            nc.scalar.dma_start(out=o_h[:, po + 64:H * W], in_=x_h[:, po + 64:H * W])
            nc.scalar.dma_start(out=o_h[:, po:po + 64], in_=zeros[:])

            # ---- identity + PE warmup (clock ramp) ----
            # Warm on the early-memset zeros tile: identity (Pool-built) is
            # only ready ~2.6us in, far too late to start the PE p-state ramp.
            ident = sb.tile([128, 128], mybir.dt.bfloat16)
            make_identity(nc, ident[:])
            warm_ps = ps.tile([64, 64], F32)
            for _ in range(2):
                nc.tensor.matmul(warm_ps[:], zeros[:, 0:64], zeros[:, 0:64],
                                 start=True, stop=True)

            # ---- delta strip: x window cols with the patch bits ----
            # big cols 61:125 = x rows ry..ry+2 from col rx (64 consecutive)
            dbase = 2 * W + rx
            delta = sb.tile([B, 75], F32)
            nc.vector.tensor_copy(out=delta[:, 0:64], in_=big[:, dbase:dbase + 64])

            # ---- scatter-add prep (descriptors only; fires via trigger) ----
            # sem must be the Tile DMASW lane this prep is assigned (it is the
            # 2nd Pool-queue DMA inst emitted -> DMASW1): the drain at context
            # exit waits on that lane, and walrus encodes exactly one DMA sem.
            import bass_rust as _br
            dma_sem = tc.sems[_br.PROC_NAMES.index("DMASW1")]
            nc.gpsimd.dma_scatter_add(
                o_h[:, po:po + 64],
                delta[:, 0:64].rearrange("b (x c) -> b x c", x=1),
                t12[:, 1024:1040].bitcast(I16),
                128,
                128,
                64,
                elem_step=OW,
                prepare_only=True,
                sem=dma_sem,
            )

            # ---- staging: extract both windows into a contiguous bf16 tile
            # (matmul stationary APs allow only one free dim, so the strided
            # window view cannot feed the transpose directly), then one
            # transpose + PSUM->SBUF convert to the matmul dtype.
            stage = sb.tile([B, CROWS], mybir.dt.bfloat16)
            nc.vector.memset(stage[:, 74:76], 1.0)  # ones rows
            bigv = big[:, 0:325].rearrange("b (r c) -> b r c", r=13)
            ws = stage[:, 0:49].rearrange("b (r c) -> b r c", r=7)
            nc.vector.tensor_copy(out=ws[:], in_=bigv[:, 0:7, rx - 2:rx + 5])
            ts = stage[:, 49:74].rearrange("b (r c) -> b r c", r=5)
            nc.vector.tensor_copy(out=ts[:], in_=bigv[:, 7:12, rx - 1:rx + 4])
            ps_stage = ps.tile([CROWS, B], mybir.dt.bfloat16)
            nc.tensor.transpose(out=ps_stage[:], in_=stage[:], identity=ident[:])
            stageT = sb.tile([CROWS, B], LP)
            nc.vector.tensor_copy(out=stageT[:], in_=ps_stage[:])

            # ---- D matmuls + one-hot G + E matmuls ----
            D12 = ps.tile([128, 2 * B], F32)
            D3 = ps.tile([TILE_SIZES[2], B], F32)
            E_ps = ps.tile([B, NPAT], F32)
            # D1/D2 share one PSUM tile as column blocks so a single is_equal
            # produces G1|G2 — fewer DVE ops, and the all-G-before-all-E
            # linear order keeps tile's sem-wait coarsening from serializing
            # G ops behind E matmuls.
            n3 = TILE_SIZES[2]
            nc.tensor.matmul(D12[:, 0:B], cgeo[:, 0:128], stageT[:],
                             start=True, stop=True)
            nc.tensor.matmul(D12[:, B:2 * B], cgeo[:, 128:256], stageT[:],
                             start=True, stop=True)
            nc.tensor.matmul(D3[:], cgeo[:, 256:256 + n3], stageT[:],
                             start=True, stop=True)
            G12 = sb.tile([128, 2 * B], LP)
            nc.vector.tensor_scalar(G12[:], D12[:], 0.0, None, OP.is_equal)
            nc.vector.tensor_scalar(G3[:, 0:B], D3[:], 0.0, None, OP.is_equal)
            # DoubleRow fp8: one matmul computes G1^T@T1 + G2^T@T2 (the
            # stationary/moving free dims each hold the two operand halves)
            # at 0.5 cycles/row.
            nc.tensor.matmul(
                E_ps[:],
                G12[:].rearrange("f (two b) -> f two b", two=2),
                t12[:, 0:2 * NPAT].bitcast(LP).rearrange(
                    "f (two n) -> f two n", two=2),
                perf_mode=mybir.MatmulPerfMode.DoubleRow,
                start=True, stop=False)
            nc.tensor.matmul(
                E_ps[:],
                G3[:].rearrange("f (two b) -> f two b", two=2),
                cgeo[0:n3, 512:1536].rearrange("f (two n) -> f two n", two=2),
                perf_mode=mybir.MatmulPerfMode.DoubleRow,
                start=False, stop=True)


            # ---- argmin via negate/max/max_index (reference tie semantics) ----
            negseed = sb.tile([B, NPAT], F32)
            nc.vector.scalar_tensor_tensor(
                out=negseed[:], in0=noise[:], scalar=-0.5, in1=E_ps[:],
                op0=OP.mult, op1=OP.subtract,
            )
            mx8 = sb.tile([B, 8], F32)
            nc.vector.max(out=mx8[:], in_=negseed[:])
            idx8 = sb.tile([B, 8], U32)
            nc.vector.max_index(out=idx8[:], in_max=mx8[:], in_values=negseed[:])

            # ---- bits of the argmin into the delta strip ----
            shd = sb.tile([B, 9], U32)
            nc.vector.tensor_tensor(
                out=shd[:], in0=idx8[:, 0:1].to_broadcast([B, 9]), in1=shvec[:],
                op=OP.logical_shift_right,
            )
            bitu = sb.tile([B, 9], U32)
            nc.vector.tensor_scalar(bitu[:], shd[:], 1, None, OP.bitwise_and)
            dview = delta[:].rearrange("b (r c) -> b r c", r=3)
            nc.vector.tensor_copy(
                out=dview[:, :, 0:3], in_=bitu[:].rearrange("b (r c) -> b r c", r=3))

            # ---- fire the patch scatter (the context-exit drain waits on
            # the DMASW1 lane, so the kernel cannot end before it lands) ----
            nc.gpsimd.trigger_dma(count=None)

    nc.finalize()
    return nc


def _get(ry, rx):
    key = (ry, rx)
    if key not in _CACHE:
        _CACHE[key] = _build(ry, rx)
    return _CACHE[key]


def kernel_with_results(x, target, noise, ry, rx, trace=False):
    x = np.ascontiguousarray(np.asarray(x, dtype=np.float32))
    target = np.ascontiguousarray(np.asarray(target, dtype=np.float32))
    noise = np.ascontiguousarray(np.asarray(noise, dtype=np.float32))
    ry, rx = int(ry), int(rx)
    Btot = x.shape[0]
    assert Btot == B_TOTAL and x.shape == (Btot, 1, H, W), x.shape

    nc = _get(ry, rx)
    xs = x.reshape(Btot, H * W)
    ts = target.reshape(Btot, H * W)
    in_maps = [
        {
            "x": xs[c * B:(c + 1) * B],
            "target": ts[c * B:(c + 1) * B],
            "noise": noise[c * B:(c + 1) * B],
        }
        for c in range(N_CORES)
    ]
    res = run_bass_kernel_spmd(nc, in_maps, core_ids=list(range(N_CORES)), trace=trace)
    out = np.concatenate(
        [res.results[c]["out"][:, :H * W] for c in range(N_CORES)], axis=0)
    return out.reshape(Btot, 1, H, W).astype(np.float32), res


def kernel(x, target, noise, ry, rx):
    out, _ = kernel_with_results(x, target, noise, ry, rx)
    return out
